# revision 1
# baseline (speedup 1.0000x reference)
"""AdaptiveConv2DMod Trainium2 kernel (v2).

Per-sample modulated 3x3 conv (StyleGAN2-style) on 8 NeuronCores,
data-parallel over batch (1 sample per core, no collectives).

Per-core layout (v2 — offset-bank scheme):
  - Input rows stream through a circular SBUF tape X: image row r lives at
    partition strip r%4 (32 channels each), slot r//4 (mod nslot), width
    padded to W+4 with zero columns.  bf16.
  - Output "bank" t covers rows 4t+1..4t+4 (offset +1 vs the tape slots).
    The 6 input rows it needs (4t..4t+5) split 4+2 across tape slots
    t, t+1 — so the whole bank is computed by 6 matmuls, all M=128:
      A1(sigma): K=128 over slot t     (9 nonzero 32x32 weight blocks)
      A2(sigma): K=64  over slot t+1 strips 0-1 (3 nonzero blocks),
    sigma = kx in {0,1,2} realized as a +sigma column offset in the rhs AP.
    vs v1's 12 M=32 matmuls per bank: half the PE streams, and the
    phase-shifted X2 tape (16.8MB of SBUF->SBUF DMA) disappears.
  - Edge banks: bank -1 (row 0 via its A2 only) and bank 127 (rows
    509..511 via its A1 only) reuse the standard weight matrices; the
    out-of-range column groups compute garbage that is never evacuated.
  - Weights (softmax-mixed + modulated + demodulated, built on device) are
    transposed into the two block layouts WA1 [128, 3*128] / WA2 [64, 3*64]
    by PE transposes into PSUM + narrow DVE copies (x (1+mod) scaling).
  - Matmuls run in bf16 (DVE converts the f32 staging tile), PSUM/out f32.
"""

import sys

import numpy as np

try:
    import concourse.bass as bass  # noqa: F401
except ImportError:
    sys.path.insert(0, "/opt/trn_rl_repo")

import concourse.bass as bass
import concourse.tile as tile
from concourse import bacc, mybir
from concourse.bass_utils import run_bass_kernel_spmd

F32 = mybir.dt.float32
BF16 = mybir.dt.bfloat16

C = 32          # in/out channels
NK = 4          # kernel bank size
EPS = 1e-8


def build_graph(H=512, W=512, nslot=64, ablate="", timing=False, repeat=1):
    """Build the per-core Bass graph. Returns compiled Bacc.

    ablate: comma-set of stages to skip ("mm", "evac", "cvt", "odma")
    for TimelineSim bottleneck analysis only.
    timing: write the image to an Internal DRAM scratch and expose only a
    tiny external output, so repeated executions can be queued back-to-back
    without device-memory pressure (wall-clock delta timing).
    """
    skip = set(ablate.split(",")) if ablate else set()
    T = H // 4                      # tape slots / banks per image
    nslot = min(nslot, T)
    Wp = W + 4                      # padded width

    nc = bacc.Bacc("TRN2", target_bir_lowering=False, debug=False)

    fmap = nc.dram_tensor("fmap", [C, H, W], F32, kind="ExternalInput")
    mod = nc.dram_tensor("mod", [1, C], F32, kind="ExternalInput")
    kmod = nc.dram_tensor("kernel_mod", [1, NK], F32, kind="ExternalInput")
    wbank = nc.dram_tensor("weights", [NK, C, C, 3, 3], F32, kind="ExternalInput")
    ident = nc.inline_tensor(np.eye(C, dtype=np.float32), name="ident32")
    if timing:
        out = nc.dram_tensor("out", [1, NK], F32, kind="ExternalOutput")
        oscr = nc.dram_tensor("oscr", [C, H, W], F32, kind="Internal")
        osink = oscr
    else:
        out = nc.dram_tensor("out", [C, H, W], F32, kind="ExternalOutput")
        osink = out

    SBLK = min(16, T)               # slots per DMA block
    NBLK = T // SBLK
    fm_bs = fmap.ap().rearrange("i (b s g) x -> b g i s x", s=SBLK, g=4)

    with tile.TileContext(nc) as tc:
        with (
            tc.tile_pool(name="xpool", bufs=1) as xpool,
            tc.tile_pool(name="wpool", bufs=1) as wpool,
            tc.tile_pool(name="cpool", bufs=1) as cpool,
        ):
            X = xpool.tile([128, nslot * Wp], BF16)
            WA1 = wpool.tile([128, 3 * 128], BF16)    # sigma-major A1 lhsT
            WA2 = wpool.tile([64, 3 * 128], BF16)     # sigma-major A2 lhsT
            id32 = cpool.tile([C, C], F32)
            ones1 = cpool.tile([1, C], F32)
            m1B = cpool.tile([128, 1], F32)           # (1+mod[ci]) tiled x4
            e0 = cpool.tile([C, W], F32)              # row 0 staging
            e1 = cpool.tile([96, W], F32)             # rows H-3..H-1 staging
            s4 = cpool.tile([1, NK], F32)             # timing-mode sink

            nc.sync.dma_start(id32[:, :], ident.ap())
            nc.gpsimd.memset(ones1[:, :], 1.0)
            # zero weight tiles: only valid blocks are copied in
            nc.vector.memset(WA1[:, :], 0.0)
            nc.vector.memset(WA2[:, :], 0.0)

            # zero the pad columns of every slot
            xv = X[:, :].rearrange("p (s q) -> p s q", q=Wp)
            nc.vector.memset(xv[:, :, 0:2], 0.0)
            nc.vector.memset(xv[:, :, Wp - 2:Wp], 0.0)

            # ---------------- weight preparation ----------------
            with (
                tc.tile_pool(name="prep", bufs=2) as prep,
                tc.tile_pool(name="prep_ps", bufs=2, space="PSUM") as prep_ps,
            ):
                # softmax(kernel_mod) -> attn [1, NK]
                km = prep.tile([1, NK], F32)
                nc.sync.dma_start(km[:, :], kmod.ap())
                mx = prep.tile([1, 1], F32)
                nc.vector.reduce_max(mx[:, :], km[:, :], axis=mybir.AxisListType.X)
                nmx = prep.tile([1, 1], F32)
                nc.scalar.mul(nmx[:, :], mx[:, :], -1.0)
                ex = prep.tile([1, NK], F32)
                nc.scalar.activation(
                    ex[:, :], km[:, :], mybir.ActivationFunctionType.Exp,
                    bias=nmx[:, 0:1],
                )
                sm = prep.tile([1, 1], F32)
                nc.vector.reduce_sum(sm[:, :], ex[:, :], axis=mybir.AxisListType.X)
                rs = prep.tile([1, 1], F32)
                nc.vector.reciprocal(rs[:, :], sm[:, :])
                attn = prep.tile([1, NK], F32)
                nc.vector.tensor_scalar_mul(attn[:, :], ex[:, :], rs[:, 0:1])

                attnB = prep.tile([C, NK], F32)
                nc.gpsimd.partition_broadcast(attnB[:, :], attn[:, :])

                # P[o, n*288 + i*9 + tap] = weights[n, o, i, ky, kx]
                P = prep.tile([C, NK * 288], F32)
                nc.sync.dma_start(
                    P[:, :], wbank.ap().rearrange("n o i ky kx -> o n (i ky kx)")
                )

                # mix[o, i*9+tap] = sum_n attn[n] * P[o, n, ...]
                mix = prep.tile([C, 288], F32, tag="mix")
                tmp = prep.tile([C, 288], F32, tag="tmp")
                nc.vector.tensor_scalar_mul(mix[:, :], P[:, 0:288], attnB[:, 0:1])
                for n in range(1, NK):
                    nc.vector.tensor_scalar_mul(
                        tmp[:, :], P[:, n * 288:(n + 1) * 288], attnB[:, n:n + 1]
                    )
                    nc.vector.tensor_add(mix[:, :], mix[:, :], tmp[:, :])

                # mvec[i, 1] = mod + 1 ;  m2 = mvec^2 ; m1B = mvec tiled x4
                mv = prep.tile([C, 1], F32, tag="mv")
                nc.sync.dma_start(mv[:, :], mod.ap().rearrange("a i -> i a"))
                m1 = prep.tile([C, 1], F32, tag="m1")
                nc.scalar.add(m1[:, :], mv[:, :], 1.0)
                m2 = prep.tile([C, 1], F32, tag="m2")
                nc.vector.tensor_mul(m2[:, :], m1[:, :], m1[:, :])
                for g in range(4):
                    nc.sync.dma_start(
                        m1B[32 * g:32 * g + 32, :], mod.ap().rearrange("a i -> i a")
                    )
                nc.scalar.add(m1B[:, :], m1B[:, :], 1.0)

                # demodulation: inv[o] = rsqrt(sum_{i,tap} (mix * m1[i])^2)
                sq = prep.tile([C, 288], F32, tag="tmp")
                nc.vector.tensor_mul(sq[:, :], mix[:, :], mix[:, :])
                s_oi = prep.tile([C, C], F32, tag="soi")
                nc.vector.reduce_sum(
                    s_oi[:, :],
                    sq[:, :].rearrange("p (i t) -> p i t", t=9),
                    axis=mybir.AxisListType.X,
                )
                ps_a = prep_ps.tile([C, C], F32, tag="psa")
                nc.tensor.transpose(ps_a[:, :], s_oi[:, :], id32[:, :])
                sT = prep.tile([C, C], F32, tag="soi")
                nc.vector.tensor_copy(sT[:, :], ps_a[:, :])

                ps_n = prep_ps.tile([1, C], F32, tag="psa")
                nc.tensor.matmul(
                    ps_n[:, :], m2[:, :], sT[:, :], start=True, stop=True
                )
                ns = prep.tile([1, C], F32, tag="ns")
                nc.vector.tensor_scalar_max(ns[:, :], ps_n[:, :], EPS)
                sqn = prep.tile([1, C], F32, tag="sqn")
                nc.scalar.sqrt(sqn[:, :], ns[:, :])
                inv = prep.tile([1, C], F32, tag="inv")
                nc.vector.reciprocal(inv[:, :], sqn[:, :])

                ps_i = prep_ps.tile([C, 1], F32, tag="psa")
                nc.tensor.transpose(ps_i[:, :], inv[:, :], ones1[:, 0:1])
                invT = prep.tile([C, 1], F32, tag="invT")
                nc.vector.tensor_copy(invT[:, :], ps_i[:, :])

                # Build gathered pre-transpose layouts in SBUF (DVE, within-
                # partition column shuffles), then transpose each out-column
                # block [32, Kblk] -> [Kblk, 32] so every matmul PSUM output
                # sits at partition 0 (walrus verifier requirement).
                # mix4 view: [o, kx, ky, ci]
                mix4 = mix[:, :].rearrange(
                    "o (i ky kx) -> o kx ky i", ky=3, kx=3)
                # wtAB block (sg, c) at col (4*sg+c)*128:
                #   cols 32*(c+ky)+ci = mix[o, ci, ky, sg] * inv[o]
                wtAB = prep.tile([C, 12 * 128], F32, name="wtAB")
                nc.vector.memset(wtAB[:, :], 0.0)
                for sg in range(3):
                    for c in range(4):
                        nky = min(3, 4 - c)
                        base = (4 * sg + c) * 128 + 32 * c
                        nc.vector.tensor_scalar_mul(
                            wtAB[:, base:base + 32 * nky].rearrange(
                                "o (ky i) -> o ky i", i=C),
                            mix4[:, sg, 0:nky, :],
                            invT[:, 0:1],
                        )
                # wtAB2 block (sg, c in {2,3}) at col 128*sg + 64*(c-2):
                #   cols 32*p'+ci = w[ky=p'+4-c] (c=2: p'=0 ky=2;
                #   c=3: p'=ky-1, ky in {1,2})
                wtAB2 = prep.tile([C, 3 * 128], F32, name="wtAB2")
                nc.vector.memset(wtAB2[:, :], 0.0)
                for sg in range(3):
                    nc.vector.tensor_scalar_mul(
                        wtAB2[:, 128 * sg:128 * sg + 32],
                        mix4[:, sg, 2, :],
                        invT[:, 0:1],
                    )
                    nc.vector.tensor_scalar_mul(
                        wtAB2[:, 128 * sg + 64:128 * sg + 128].rearrange(
                            "o (ky i) -> o ky i", i=C),
                        mix4[:, sg, 1:3, :],
                        invT[:, 0:1],
                    )
                # PE transposes: wide blocks -> PSUM at partition 0
                pw1 = prep_ps.tile([128, 3 * 128], F32, tag="pw1")
                tps = [(sg, c) for sg in range(3) for c in range(4)]
                for i, (sg, c) in enumerate(tps):
                    nc.tensor.matmul(
                        pw1[:, 128 * sg + 32 * c:128 * sg + 32 * c + 32],
                        wtAB[:, (4 * sg + c) * 128:(4 * sg + c) * 128 + 128],
                        id32[:, :],
                        is_transpose=True,
                        start=(i == 0), stop=(i == len(tps) - 1),
                    )
                pw2 = prep_ps.tile([64, 3 * 128], F32, tag="pw2")
                tps2 = [(sg, c) for sg in range(3) for c in (2, 3)]
                for i, (sg, c) in enumerate(tps2):
                    nc.tensor.matmul(
                        pw2[:, 128 * sg + 32 * c:128 * sg + 32 * c + 32],
                        wtAB2[:, 128 * sg + 64 * (c - 2):
                              128 * sg + 64 * (c - 2) + 64],
                        id32[:, :],
                        is_transpose=True,
                        start=(i == 0), stop=(i == len(tps2) - 1),
                    )
                # evacuate to bf16 weight tiles, scaled by (1+mod[ci])
                for sg in range(3):
                    nc.vector.tensor_scalar_mul(
                        WA1[:, 128 * sg:128 * sg + 128],
                        pw1[:, 128 * sg:128 * sg + 128],
                        m1B[:, 0:1],
                    )
                    nc.vector.tensor_scalar_mul(
                        WA2[0:64, 128 * sg + 64:128 * sg + 128],
                        pw2[0:64, 128 * sg + 64:128 * sg + 128],
                        m1B[0:64, 0:1],
                    )

            # ---------------- main conv loop ----------------
            with (
                tc.tile_pool(name="cps", bufs=8, space="PSUM") as cps,
                tc.tile_pool(name="opool", bufs=2) as opool,
                tc.tile_pool(name="spool", bufs=2) as spool,
            ):
                def load_block(b):
                    stg = spool.tile([128, SBLK * W], F32, tag="stg")
                    for g in range(4):
                        nc.sync.dma_start(
                            stg[32 * g:32 * g + 32, :].rearrange(
                                "p (s x) -> p s x", x=W
                            ),
                            fm_bs[b, g],
                        )
                    return stg

                def convert_block(stg, b, lo=0, nsl=None):
                    if "cvt" in skip:
                        return
                    if nsl is None:
                        nsl = SBLK
                    for j in range(lo, lo + nsl, 2):
                        p = (b * SBLK + j) % nslot
                        nc.vector.tensor_copy(
                            xv[:, p:p + 2, 2:2 + W],
                            stg[:, j * W:(j + 2) * W].rearrange(
                                "p (s x) -> p s x", x=W
                            ),
                        )

                def load_half(stg, lo, nsl):
                    for g in range(4):
                        nc.sync.dma_start(
                            stg[32 * g:32 * g + 32,
                                lo * W:(lo + nsl) * W].rearrange(
                                "p (s x) -> p s x", x=W
                            ),
                            fm_bs[0, g, :, lo:lo + nsl, :],
                        )

                def mm_bank(pts_t, t, sigma, which, first, last):
                    # one stream for bank t (out rows 4t+1..4t+4)
                    if which == 1:          # A1: K=128 over slot t
                        st = t % nslot
                        lhsT = WA1[:, 128 * sigma:128 * sigma + 128]
                        rhs = X[0:128, st * Wp + 1 + sigma:
                                st * Wp + 1 + sigma + W]
                        o = pts_t[:, :]
                    else:                   # A2: K=64 over slot t+1 strips 0-1
                        s1 = (t + 1) % nslot
                        lhsT = WA2[0:64, 128 * sigma:128 * sigma + 128]
                        rhs = X[0:64, s1 * Wp + 1 + sigma:
                                s1 * Wp + 1 + sigma + W]
                        o = pts_t[:, :]
                    nc.tensor.matmul(
                        o, lhsT, rhs, start=first, stop=last,
                        skip_group_check=True,
                    )

                for _rep in range(repeat):
                    stg0 = spool.tile([128, SBLK * W], F32, tag="stg")
                    half = max(SBLK // 2, 1)
                    load_half(stg0, 0, half)
                    convert_block(stg0, 0, 0, half)
                    if half < SBLK:
                        load_half(stg0, half, SBLK - half)
                        convert_block(stg0, 0, half, SBLK - half)
                    if NBLK > 1:
                        stg1 = load_block(1)
                        convert_block(stg1, 1)

                    otiles = {}
                    next_load = 2
                    OB = 8                  # banks per output half-block
                    NG = T // 4             # 32 groups of 4 banks
                    for k in range(NG):
                        banks = [t for t in range(4 * k - 1, 4 * k + 3)
                                 if t < T - 1]
                        if k % 2 == 0:
                            h = k // 2
                            otiles[h] = opool.tile(
                                [128, OB * W], F32, name=f"ot{h}", tag="ot")
                        cur_blk = max(4 * k - 1, 0) // SBLK
                        if k % 2 == 1 and next_load < NBLK \
                                and next_load <= cur_blk + 3:
                            stgb = load_block(next_load)
                            convert_block(stgb, next_load)
                            next_load += 1

                        if "mm" not in skip:
                            pts = {t: cps.tile([128, W], F32,
                                               name=f"pt{t}", tag="pt")
                                   for t in banks}
                            # order: A1s0 (start, full bank) -> A2s (interior
                            # accumulates) -> A1s1 -> A1s2 (stop, full bank).
                            # The closing matmul must span all 128 partitions
                            # so every started psum group is closed.
                            for t in banks:
                                if t >= 0:
                                    mm_bank(pts[t], t, 0, 1,
                                            first=True, last=False)
                            for sigma in range(3):
                                for t in banks:
                                    mm_bank(pts[t], t, sigma, 2,
                                            first=(t < 0 and sigma == 0),
                                            last=(t < 0 and sigma == 2))
                            for sigma in (1, 2):
                                for t in banks:
                                    if t >= 0:
                                        mm_bank(pts[t], t, sigma, 1,
                                                first=False,
                                                last=(sigma == 2))

                        # evacuations (ACT) + per-block output DMA (SWDGE)
                        for t in banks:
                            if "evac" in skip:
                                continue
                            if t < 0:
                                nc.scalar.copy(e0[:, :], pts[t][96:128, :])
                                if "odma" not in skip:
                                    nc.gpsimd.dma_start(
                                        osink.ap()[:, 0, :],
                                        e0[:, :],
                                    )
                            else:
                                hw_ = (t % OB) * W
                                nc.scalar.copy(
                                    otiles[t // OB][:, hw_:hw_ + W],
                                    pts[t][:, :])
                        if k % 2 == 0 and k > 0 and "odma" not in skip \
                                and "evac" not in skip:
                            hdone = k // 2 - 1
                            _emit_half_dma(nc, osink, otiles[hdone], hdone,
                                           OB, W)
                            otiles.pop(hdone, None)

                    # tail: bank T-1 (rows H-3..H-1) + last block's DMA
                    if "mm" not in skip:
                        ptl = cps.tile([128, W], F32, name="ptl", tag="pt")
                        for sigma in range(3):
                            mm_bank(ptl, T - 1, sigma, 1,
                                    first=(sigma == 0), last=(sigma == 2))
                    if "evac" not in skip:
                        if "odma" not in skip:
                            hlast = T // OB - 1
                            _emit_half_dma(nc, osink, otiles[hlast], hlast,
                                           OB - 1, W)
                            otiles.pop(hlast, None)
                        nc.scalar.copy(e1[:, :], ptl[0:96, :])
                        if "odma" not in skip:
                            for c in range(3):
                                nc.gpsimd.dma_start(
                                    osink.ap()[:, H - 3 + c, :],
                                    e1[32 * c:32 * c + 32, :],
                                )

                if timing:
                    nc.sync.dma_start(s4[:, :], osink.ap()[0:1, 0, 0:NK])
                    nc.sync.dma_start(out.ap(), s4[:, :])

    nc.compile()
    return nc


def _emit_half_dma(nc, osink, ot, h, nslots, W):
    """Output DMA for half-block h (8 banks): rows 32h+1 .. 32h+1+4*nslots."""
    r0 = 32 * h + 1
    ov = osink.ap()[:, r0:r0 + 4 * nslots, :].rearrange(
        "o (s c) x -> c o s x", s=nslots, c=4
    )
    for c in range(4):
        nc.gpsimd.dma_start(
            ov[c],
            ot[32 * c:32 * c + 32, 0:nslots * W].rearrange(
                "p (s x) -> p s x", x=W
            ),
        )


_CACHE = {}


def _get_graph(H, W):
    key = (H, W)
    if key not in _CACHE:
        _CACHE[key] = build_graph(H, W)
    return _CACHE[key]


def kernel(fmap, mod, kernel_mod, weights):
    B, Ci, H, Wd = fmap.shape
    nc = _get_graph(H, Wd)
    in_maps = [
        {
            "fmap": np.ascontiguousarray(fmap[b], dtype=np.float32),
            "mod": np.ascontiguousarray(mod[b:b + 1], dtype=np.float32),
            "kernel_mod": np.ascontiguousarray(kernel_mod[b:b + 1], dtype=np.float32),
            "weights": np.ascontiguousarray(weights, dtype=np.float32),
        }
        for b in range(B)
    ]
    res = run_bass_kernel_spmd(nc, in_maps, core_ids=list(range(B)))
    return np.stack([res.results[b]["out"] for b in range(B)], axis=0)

